# revision 24
# baseline (speedup 1.0000x reference)
"""DetectionBEVLoss Trainium2 kernel: 8-core data-parallel (1 batch/core).

v2 design:
- Host compacts w>0 elements (geometry/sl1/bce run on [128, GW=272] instead
  of [128,512]); zero-padding contributes exactly 0 to every masked sum.
  Focal runs dense on all 65536 elements/core.
- Rotated IoU via midpoint Liang-Barsky: per box pair, 8 edge-pair-coords
  (slab, mbar, off, rho) built directly from center/trig products -- no
  corner tensors. Green's-theorem integral with constant-cross trick for
  the target-box direction.
- Custom fused DVE ops (8-deep ALU pipeline @ 1 elem/cycle/lane):
  2-NR reciprocal (stock RECIPROCAL_APPROX_FAST), seg=relu(min(H,1)+min(L,1)),
  fused smooth-L1+accumulate, clamped square-sum.
- ACT: sin/cos (table), exp, ln, abs, square, accumulations.
  Pool: class-sum avgpool for softmax denom + enclosing-box min/max chain.
- Host packs x_t = cls_pred[cls_t] (pure gather) so focal needs no
  10-way mask reduction on device.
"""
import math
import operator

import numpy as np

import concourse.bacc as bacc
import concourse.bass as bass
import concourse.mybir as mybir
import concourse.tile as tile
from concourse.bass_utils import run_bass_kernel_spmd

F16 = mybir.dt.float16
F32 = mybir.dt.float32
OP = mybir.AluOpType
AF = mybir.ActivationFunctionType

P = 128
FW = 512          # full free width (focal)
GW = 264          # compacted geometry width (33792 slots; ~32768 positives)
NG = 21           # geometry slots

# ---------------------------------------------------------------------------
# custom DVE ops: register into the concourse op table at import time.
# ---------------------------------------------------------------------------
from concourse import dve_ops as _dve_ops
from concourse.dve_ops import (
    DveOp,
    OPS as _OPS,
    RECIPROCAL_APPROX_FAST,
    RECIP_APPROX_FAST_CONSTS,
    _SUB_OPCODE_FOR_NAME,
    CUSTOM_DVE_SPECS,
)
from concourse.dve_spec import (
    Spec, Src0, Src1, C0, C2, One, Bin, AluOp, relu, sq, maxx, minn, lower,
    _has_src1,
)
from concourse.dve_uop import DveOpSpec


def _register(name, spec, subdim=False):
    if name in _SUB_OPCODE_FOR_NAME:
        return next(o for o in _OPS if o.name == name)
    row = max(_SUB_OPCODE_FOR_NAME.values()) + 1
    assert row < 0x20, "custom DVE opcode rows exhausted"
    uops = lower(spec, ver="v3")
    sp = DveOpSpec(name=name, opcode=row, uops=uops, rd1_en=_has_src1(spec))
    op = DveOp(name, spec, subdim=subdim, uops_sha={"v3": sp.sha("v3")})
    _OPS.append(op)
    _SUB_OPCODE_FOR_NAME[name] = row
    CUSTOM_DVE_SPECS[name] = spec
    return op


def _dve_minmax(a, b, is_min):
    # DVE MIN/MAX return the non-NaN operand
    a2 = np.where(np.isnan(a), b, a)
    b2 = np.where(np.isnan(b), a, b)
    return np.minimum(a2, b2) if is_min else np.maximum(a2, b2)


def _segrel_ref(in0, in1, s0, s1, imm2):
    m1 = _dve_minmax(in0.astype(np.float32), 1.0, True)
    m2 = _dve_minmax(in1.astype(np.float32), 1.0, True)
    return _dve_minmax(m1 + m2, 0.0, False)


def _sl1acc_ref(in0, in1, s0, s1, imm2):
    ad = np.abs(in0.astype(np.float32) - in1.astype(np.float32))
    m = np.minimum(ad, 1.0)
    b = ad * m + s0 * m * m
    return b, b.reshape(b.shape[0], -1).sum(axis=-1, keepdims=True)


def _sqsumm_ref(in0, in1, s0, s1, imm2):
    return np.maximum(in0.astype(np.float32) ** 2 + in1.astype(np.float32) ** 2,
                      imm2)


def _bce1_ref(in0, in1, s0, s1, imm2):
    x = in0.astype(np.float32)
    return np.maximum(x, 0) - x * in1.astype(np.float32)


_ad = Bin(AluOp.ABSOLUTE_DIFF, Src0, Src1)
_m = minn(_ad, One)
SEGREL = _register(
    "SEGREL_ANT",
    Spec(body=relu(minn(Src0, One) + minn(Src1, One)), reference=_segrel_ref))
SL1ACC = _register(
    "SL1ACC_ANT",
    Spec(body=_ad * _m + sq(_m) * C0, accum=operator.add,
         reference=_sl1acc_ref))
SQSUMM = _register(
    "SQSUMM_ANT",
    Spec(body=maxx(sq(Src0) + sq(Src1), C2), reference=_sqsumm_ref))
BCE1 = _register(
    "BCE1_ANT",
    Spec(body=relu(Src0) - Src0 * Src1, reference=_bce1_ref))
ABSMUL = _register(
    "ABSMUL_ANT",
    Spec(body=Bin(AluOp.ABSOLUTE_VALUE, Src0, Src0) * C0,
         reference=lambda in0, in1, s0, s1, imm2:
             np.abs(in0.astype(np.float32)) * s0))

RECIP_K = dict(s0=RECIP_APPROX_FAST_CONSTS["s0"],
               s1=RECIP_APPROX_FAST_CONSTS["s1"],
               imm2=RECIP_APPROX_FAST_CONSTS["imm2"])


def _ap(t, s0, slot_dims, col0, ncol, colstep=1):
    """Manual AP into tile t ([128, S, W]): base slot s0, then
    (slot_step, count) dims, innermost column dim."""
    ss = t.ap[-2][0]
    ap = [list(t.ap[0])] + [[s * ss, c] for s, c in slot_dims] + [[colstep, ncol]]
    return bass.AP(tensor=t.tensor, offset=t.offset + s0 * ss + col0, ap=ap)


def build_bass():
    nc = bacc.Bacc("TRN2", target_bir_lowering=False, debug=False)
    g = nc.declare_dram_parameter("g", [P, NG, GW], F16, isOutput=False)
    f10 = nc.declare_dram_parameter("f10", [P, 10, FW], F16, isOutput=False)
    xtc = nc.declare_dram_parameter("xtc", [P, 2, FW], F16, isOutput=False)
    outp = nc.declare_dram_parameter("out", [1, 8], F32, isOutput=True)

    with tile.TileContext(nc, pool_alloc_mode="queue") as tc:
        with (
            tc.tile_pool(name="main", bufs=1) as pool,
            tc.tile_pool(name="small", bufs=1) as spool,
            tc.tile_pool(name="ps", bufs=1, space="PSUM") as ppool,
        ):
            G = pool.tile([P, NG, GW], F16)
            F10 = pool.tile([P, 10, FW], F16)
            XTC = pool.tile([P, 2, FW], F16)
            nc.sync.dma_start(out=G[:, 0:6, :], in_=g[:, 0:6, :])
            nc.sync.dma_start(out=G[:, 6:NG, :], in_=g[:, 6:NG, :])
            nc.sync.dma_start(out=F10, in_=f10[:, :, :])
            nc.sync.dma_start(out=XTC, in_=xtc[:, :, :])

            ones = spool.tile([P, 1], F32)
            nc.vector.memset(ones, 1.0)
            ACC = spool.tile([P, 8], F32)
            nc.vector.memset(ACC, 0.0)

            def const_col(val):
                t = spool.tile([P, 1], F32)
                nc.vector.memset(t, val)
                return t

            LNIN = pool.tile([P, 2, FW], F16)
            LNOUT = pool.tile([P, 2, FW], F16)
            nc.vector.memset(LNIN[:, 1, :], 1.0)
            HALFPI = const_col(math.pi / 2)
            ONE_C = const_col(1.0)

            # ---------------- DVE: d3 = (dx, dy, dth) ----------------
            D3 = pool.tile([P, 3, GW], F16)
            nc.vector.tensor_tensor(out=D3, in0=G[:, 0:3, :], in1=G[:, 3:6, :],
                                    op=OP.subtract)

            # G-only work: fills DVE while ACT does trig
            SCR = pool.tile([P, 2, GW], F16)
            nc.vector._custom_dve(SL1ACC, out=SCR[:, 0, :], in0=G[:, 10, :],
                                  in1=G[:, 14, :], s0=-0.5,
                                  accum_out=ACC[:, 2:3])
            nc.vector._custom_dve(SL1ACC, out=SCR[:, 0, :], in0=G[:, 11, :],
                                  in1=G[:, 15, :], s0=-0.5,
                                  accum_out=ACC[:, 3:4])
            nc.vector._custom_dve(SL1ACC, out=SCR, in0=G[:, 12:14, :],
                                  in1=G[:, 16:18, :], s0=-0.5,
                                  accum_out=ACC[:, 4:5])
            D2T = pool.tile([P, GW], F16)
            nc.vector._custom_dve(SQSUMM, out=D2T, in0=D3[:, 0, :],
                                  in1=D3[:, 1, :], imm2=0.0)
            XIO = G[:, 18, :]
            T1B = pool.tile([P, GW], F16)
            nc.vector._custom_dve(BCE1, out=T1B, in0=XIO, in1=G[:, 19, :])

            # ---------------- ACT: trig ----------------
            # SC6 = [sp, st, sd, cp, ct, cd]
            SC6 = pool.tile([P, 6, GW], F16)
            TH2 = _ap(G, 2, [(3, 2)], 0, GW)          # (thp, tht)
            nc.scalar.activation(SC6[:, 0:2, :], TH2, AF.Sin)
            nc.scalar.activation(SC6[:, 2, :], D3[:, 2, :], AF.Sin)
            nc.scalar.activation(SC6[:, 3:5, :], TH2, AF.Sin, bias=HALFPI)
            nc.scalar.activation(SC6[:, 5, :], D3[:, 2, :], AF.Sin,
                                 bias=HALFPI)
            # ABS4 = [|cp|, |sp|, |ct|, |st|]
            ABS4 = pool.tile([P, 4, GW], F16)
            nc.scalar.activation(ABS4, _ap(SC6, 3, [(1, 2), (-3, 2)], 0, GW),
                                 AF.Abs)

            # ---------------- ACT: focal exp ----------------
            ET = pool.tile([P, 10, FW], F16)
            nc.scalar.activation(ET, F10, AF.Exp)
            ONEB = spool.tile([P, 1], F32)
            nc.vector.tensor_scalar(out=ONEB, in0=ET[:, 0, 0:1], scalar1=0.0,
                                    scalar2=1.0, op0=OP.mult, op1=OP.add)
            BA = pool.tile([P, GW], F16)
            nc.scalar.activation(BA, XIO, AF.Abs)
            nc.scalar.activation(BA, BA, AF.Exp, scale=-1.0)
            nc.scalar.activation(BA, BA, AF.Ln, bias=ONE_C, scale=ONEB)
            BJNK = pool.tile([P, GW], F16)
            nc.vector.tensor_tensor(out=T1B, in0=T1B, in1=BA, op=OP.add)
            nc.scalar.activation(BJNK, T1B, AF.Copy, accum_out=ACC[:, 5:6])
            nc.scalar.activation(BJNK, G[:, 20, :], AF.Copy,
                                 accum_out=ACC[:, 6:7])

            # ---------------- DVE: frames ----------------
            # FP8 = [ct*dx, st*dx, ct*dy, st*dy, cp*dx, sp*dx, cp*dy, sp*dy]
            FP8 = pool.tile([P, 8, GW], F16)
            DDUP = _ap(D3, 0, [(1, 2), (0, 2)], 0, GW)      # [dx, dx, dy, dy]
            nc.vector.tensor_tensor(
                out=FP8[:, 0:4, :],
                in0=_ap(SC6, 4, [(0, 2), (-3, 2)], 0, GW),   # [ct, st, ct, st]
                in1=DDUP, op=OP.mult)
            nc.vector.tensor_tensor(
                out=FP8[:, 4:8, :],
                in0=_ap(SC6, 3, [(0, 2), (-3, 2)], 0, GW),   # [cp, sp, cp, sp]
                in1=DDUP, op=OP.mult)
            # CB4 = [cBx, cBy, eAx, eAy]
            CB4 = pool.tile([P, 4, GW], F16)
            nc.vector.tensor_tensor(out=_ap(CB4, 0, [(2, 2)], 0, GW),
                                    in0=_ap(FP8, 0, [(4, 2)], 0, GW),
                                    in1=_ap(FP8, 3, [(4, 2)], 0, GW), op=OP.add)
            nc.vector.tensor_tensor(out=_ap(CB4, 1, [(2, 2)], 0, GW),
                                    in0=_ap(FP8, 2, [(4, 2)], 0, GW),
                                    in1=_ap(FP8, 1, [(4, 2)], 0, GW),
                                    op=OP.subtract)

            # ---------------- DVE: p8 = edge half-vector components ----------
            # [a1cd, a1sd, -b1sd, b1cd, a2cd, -a2sd, b2sd, b2cd]
            DIM4 = pool.tile([P, 4, GW], F16)
            nc.vector.tensor_scalar(out=DIM4, in0=G[:, 6:10, :], scalar1=0.5,
                                    scalar2=None, op0=OP.mult)
            P8 = pool.tile([P, 8, GW], F16)
            CDb = _ap(SC6, 5, [(0, 2)], 0, GW)
            SDb = _ap(SC6, 2, [(0, 2)], 0, GW)
            DIMV = _ap(DIM4, 0, [(2, 2), (1, 2)], 0, GW)
            CDb2 = _ap(SC6, 5, [(0, 2), (0, 2)], 0, GW)
            SDb2 = _ap(SC6, 2, [(0, 2), (0, 2)], 0, GW)
            nc.vector.tensor_tensor(out=_ap(P8, 0, [(4, 2), (3, 2)], 0, GW),
                                    in0=DIMV, in1=CDb2, op=OP.mult)
            nc.vector.tensor_tensor(out=_ap(P8, 1, [(4, 2), (1, 2)], 0, GW),
                                    in0=DIMV, in1=SDb2, op=OP.mult)
            NEGV = _ap(P8, 2, [(3, 2)], 0, GW)
            nc.vector.tensor_scalar(out=NEGV, in0=NEGV, scalar1=-1.0,
                                    scalar2=None, op0=OP.mult)

            # ---------------- enclosing box / c2 / d2 (early) ----------------
            E8 = pool.tile([P, 8, GW], F16)
            nc.vector.tensor_tensor(
                out=E8, in0=_ap(DIM4, 0, [(2, 2), (0, 2), (1, 2)], 0, GW),
                in1=_ap(ABS4, 0, [(1, 4), (0, 2)], 0, GW), op=OP.mult)
            ES4 = pool.tile([P, 4, GW], F16)
            nc.vector.tensor_tensor(out=ES4, in0=_ap(E8, 0, [(2, 4)], 0, GW),
                                    in1=_ap(E8, 3, [(4, 2), (-2, 2)], 0, GW),
                                    op=OP.add)
            CEN = _ap(G, 0, [(3, 2), (1, 2)], 0, GW)    # [xp, yp, xt, yt]
            XE4 = pool.tile([P, 4, GW], F16)
            XD4 = pool.tile([P, 4, GW], F16)
            nc.vector.tensor_tensor(out=XE4, in0=CEN, in1=ES4, op=OP.add)
            nc.vector.tensor_tensor(out=XD4, in0=CEN, in1=ES4, op=OP.subtract)
            HX2 = pool.tile([P, 2, GW], F16)
            LX2 = pool.tile([P, 2, GW], F16)
            nc.vector.tensor_tensor(out=HX2, in0=XE4[:, 0:2, :],
                                    in1=XE4[:, 2:4, :], op=OP.max)
            nc.vector.tensor_tensor(out=LX2, in0=XD4[:, 0:2, :],
                                    in1=XD4[:, 2:4, :], op=OP.min)
            SP2 = pool.tile([P, 2, GW], F16)
            nc.vector.tensor_tensor(out=SP2, in0=HX2, in1=LX2, op=OP.subtract)
            C2C = pool.tile([P, GW], F16)
            nc.vector._custom_dve(SQSUMM, out=C2C, in0=SP2[:, 0, :],
                                  in1=SP2[:, 1, :], imm2=6e-5)
            RC2 = pool.tile([P, GW], F16)
            nc.vector._custom_dve(RECIPROCAL_APPROX_FAST, out=RC2, in0=C2C,
                                  **RECIP_K)
            DLT = pool.tile([P, GW], F16)
            nc.vector.tensor_tensor(out=DLT, in0=D2T, in1=RC2, op=OP.mult)

            JNK = pool.tile([P, GW], F16)

            # ---------------- DVE: reciprocals (2-NR) + clamp ----------------
            R8 = pool.tile([P, 8, GW], F16)
            nc.vector._custom_dve(RECIPROCAL_APPROX_FAST, out=R8, in0=P8,
                                  **RECIP_K)
            # min-first so NaN (from 1/0) lands at +8000
            nc.vector.tensor_scalar(out=R8, in0=R8, scalar1=8000.0,
                                    scalar2=-8000.0, op0=OP.min, op1=OP.max)

            # ---------------- ACT: |rho| (before ET so DVE isn't blocked) ----
            AR8 = pool.tile([P, 8, GW], F16)
            nc.scalar.activation(AR8, R8, AF.Abs)


            # ---------------- Pool: enclosing box + class-sum ----------------
            # (emitted later, after deps are defined)

            # ---------------- DVE: alpha/gamma/delta ----------------
            AL8 = pool.tile([P, 8, GW], F16)
            nc.vector.tensor_tensor(
                out=AL8, in0=_ap(DIM4, 2, [(-2, 2), (0, 2), (1, 2)], 0, GW),
                in1=AR8, op=OP.mult)
            GM8 = pool.tile([P, 8, GW], F16)
            nc.vector.tensor_tensor(
                out=GM8, in0=_ap(CB4, 0, [(2, 2), (0, 2), (1, 2)], 0, GW),
                in1=R8, op=OP.mult)
            DL8 = pool.tile([P, 8, GW], F16)
            nc.vector.tensor_tensor(
                out=DL8, in0=_ap(P8, 2, [(4, 2), (-2, 2), (1, 2)], 0, GW),
                in1=R8, op=OP.mult)

            A1T = pool.tile([P, 8, GW], F16)
            A2T = pool.tile([P, 8, GW], F16)
            nc.vector.tensor_tensor(out=A1T, in0=AL8, in1=GM8, op=OP.subtract)
            nc.vector.tensor_tensor(out=A2T, in0=AL8, in1=GM8, op=OP.add)
            HT = pool.tile([P, 16, GW], F16)
            LT = pool.tile([P, 16, GW], F16)
            nc.vector.tensor_tensor(out=HT[:, 0:8, :], in0=A1T, in1=DL8,
                                    op=OP.subtract)
            nc.vector.tensor_tensor(out=HT[:, 8:16, :], in0=A1T, in1=DL8,
                                    op=OP.add)
            nc.vector.tensor_tensor(out=LT[:, 0:8, :], in0=A2T, in1=DL8,
                                    op=OP.add)
            nc.vector.tensor_tensor(out=LT[:, 8:16, :], in0=A2T, in1=DL8,
                                    op=OP.subtract)

            SH8 = pool.tile([P, 8, GW], F16)
            SL8 = pool.tile([P, 8, GW], F16)
            nc.vector.tensor_tensor(out=SH8, in0=_ap(HT, 0, [(2, 8)], 0, GW),
                                    in1=_ap(HT, 1, [(2, 8)], 0, GW), op=OP.min)
            nc.vector.tensor_tensor(out=SL8, in0=_ap(LT, 0, [(2, 8)], 0, GW),
                                    in1=_ap(LT, 1, [(2, 8)], 0, GW), op=OP.min)
            SEG8 = pool.tile([P, 8, GW], F16)
            nc.vector._custom_dve(SEGREL, out=SEG8, in0=SH8, in1=SL8)

            # ---------------- DVE: focal chain (early, overlaps clip) ----
            T5 = pool.tile([P, 5, FW], F16)
            nc.vector.tensor_tensor(out=T5, in0=ET[:, 0:5, :],
                                    in1=ET[:, 5:10, :], op=OP.add)
            T2B = pool.tile([P, 2, FW], F16)
            nc.vector.tensor_tensor(out=T2B, in0=T5[:, 0:2, :],
                                    in1=T5[:, 2:4, :], op=OP.add)
            SAVG = pool.tile([P, FW], F16)
            nc.vector.tensor_tensor(out=SAVG, in0=T2B[:, 0, :],
                                    in1=T2B[:, 1, :], op=OP.add)
            nc.vector.tensor_tensor(out=SAVG, in0=SAVG, in1=T5[:, 4, :],
                                    op=OP.add)
            LNS = pool.tile([P, FW], F16)
            nc.scalar.activation(LNS, SAVG, AF.Ln)
            nc.vector.tensor_tensor(out=LNOUT[:, 0, :], in0=XTC[:, 0, :],
                                    in1=LNS, op=OP.subtract)
            PTT = pool.tile([P, FW], F16)
            nc.scalar.activation(PTT, LNOUT[:, 0, :], AF.Exp)
            OM2 = pool.tile([P, FW], F16)
            nc.scalar.activation(OM2, PTT, AF.Square, scale=-1.0,
                                 bias=ONE_C)
            ALPT = pool.tile([P, FW], F16)
            nc.vector.tensor_scalar(out=ALPT, in0=XTC[:, 1, :], scalar1=0.5,
                                    scalar2=-0.5, op0=OP.is_gt, op1=OP.mult)
            FF = pool.tile([P, FW], F16)
            nc.vector.tensor_tensor(out=FF, in0=OM2, in1=LNOUT[:, 0, :],
                                    op=OP.mult)
            nc.vector.scalar_tensor_tensor(out=FF, in0=ALPT, scalar=0.75,
                                           in1=FF, op0=OP.add, op1=OP.mult)
            JNKF = pool.tile([P, FW], F16)
            nc.scalar.activation(JNKF, FF, AF.Copy, scale=-1.0,
                                 accum_out=ACC[:, 0:1])


            # ---------------- DVE: integral ----------------
            PS4 = pool.tile([P, 4, GW], F16)
            nc.vector.tensor_tensor(out=PS4, in0=SEG8[:, 0:4, :],
                                    in1=SEG8[:, 4:8, :], op=OP.add)
            PD2 = pool.tile([P, 2, GW], F16)
            nc.vector.tensor_tensor(out=PD2, in0=SEG8[:, 0:2, :],
                                    in1=SEG8[:, 4:6, :], op=OP.subtract)
            SAB2 = pool.tile([P, 2, GW], F16)
            nc.vector.tensor_tensor(out=SAB2, in0=_ap(PS4, 0, [(2, 2)], 0, GW),
                                    in1=_ap(PS4, 1, [(2, 2)], 0, GW), op=OP.add)
            CP4 = pool.tile([P, 4, GW], F16)
            nc.vector.tensor_tensor(out=CP4,
                                    in0=_ap(CB4, 0, [(0, 2), (1, 2)], 0, GW),
                                    in1=_ap(P8, 1, [(2, 2), (-1, 2)], 0, GW),
                                    op=OP.mult)
            CX2 = pool.tile([P, 2, GW], F16)
            nc.vector.tensor_tensor(out=CX2, in0=_ap(CP4, 0, [(2, 2)], 0, GW),
                                    in1=_ap(CP4, 1, [(2, 2)], 0, GW),
                                    op=OP.subtract)
            M2 = pool.tile([P, 2, GW], F16)
            nc.vector.tensor_tensor(out=M2, in0=CX2, in1=PD2, op=OP.mult)
            AB2 = pool.tile([P, 2, GW], F16)
            nc.vector.tensor_tensor(out=AB2, in0=_ap(DIM4, 0, [(2, 2)], 0, GW),
                                    in1=_ap(DIM4, 1, [(2, 2)], 0, GW),
                                    op=OP.mult)
            IAB2 = pool.tile([P, 2, GW], F16)
            nc.vector.tensor_tensor(out=IAB2, in0=AB2, in1=SAB2, op=OP.mult)
            IA1 = pool.tile([P, GW], F16)
            nc.vector.tensor_tensor(out=IA1, in0=M2[:, 1, :], in1=M2[:, 0, :],
                                    op=OP.subtract)
            nc.vector.tensor_tensor(out=IA1, in0=IA1, in1=IAB2[:, 0, :],
                                    op=OP.add)
            nc.vector.tensor_tensor(out=IA1, in0=IA1, in1=IAB2[:, 1, :],
                                    op=OP.add)
            INTER = pool.tile([P, GW], F16)
            nc.vector._custom_dve(ABSMUL, out=INTER, in0=IA1, s0=0.5)

            # ---------------- DVE: union + iou ----------------
            USUM = pool.tile([P, GW], F16)
            nc.vector.tensor_tensor(out=USUM, in0=AB2[:, 0, :],
                                    in1=AB2[:, 1, :], op=OP.add)
            U = pool.tile([P, GW], F16)
            nc.vector.scalar_tensor_tensor(out=U, in0=USUM, scalar=4.0,
                                           in1=INTER, op0=OP.mult,
                                           op1=OP.subtract)
            nc.vector.tensor_scalar(out=U, in0=U, scalar1=6e-5, scalar2=None,
                                    op0=OP.max)
            RU = pool.tile([P, GW], F16)
            nc.vector._custom_dve(RECIPROCAL_APPROX_FAST, out=RU, in0=U,
                                  **RECIP_K)
            IOU = pool.tile([P, GW], F16)
            nc.vector.tensor_tensor(out=IOU, in0=INTER, in1=RU, op=OP.mult)
            nc.vector.tensor_tensor(out=DLT, in0=DLT, in1=IOU, op=OP.subtract)
            nc.scalar.activation(JNK, DLT, AF.Copy, accum_out=ACC[:, 1:2])





            # ---------------- cross-partition reduce + out ----------------
            PS = ppool.tile([1, 8], F32)
            nc.tensor.matmul(PS, ones, ACC, start=True, stop=True)
            OUT = spool.tile([1, 8], F32)
            nc.scalar.copy(out=OUT, in_=PS)
            nc.sync.dma_start(out=outp[:, :], in_=OUT)
    nc.compile()
    return nc


_NC_CACHE = None


def _get_nc():
    global _NC_CACHE
    if _NC_CACHE is None:
        _NC_CACHE = build_bass()
    return _NC_CACHE


def pack_inputs(cls_pred, reg_pred, iou_pred, reg_targets, iou_targets,
                cls_targets, reg_weights):
    """Returns list of 8 per-core input dicts."""
    B = cls_pred.shape[0]
    N = FW * P
    maps = []
    for b in range(B):
        rp = np.asarray(reg_pred[b], np.float32).reshape(9, N)
        rt = np.asarray(reg_targets[b], np.float32).reshape(9, N)
        ip = np.asarray(iou_pred[b], np.float32).reshape(N)
        it = np.asarray(iou_targets[b], np.float32).reshape(N)
        cf = np.asarray(cls_targets[b]).reshape(N)
        wf = np.asarray(reg_weights[b]).reshape(N)

        pos = np.flatnonzero(wf > 0)
        npos = pos.size
        assert npos <= P * GW, f"core {b}: {npos} positives > {P * GW}"

        gh = np.zeros((NG, P * GW), np.float16)
        sel = [rp[0], rp[1], rp[6], rt[0], rt[1], rt[6],
               rp[4], rp[3], rt[4], rt[3],
               rp[2], rp[5], rp[7], rp[8],
               rt[2], rt[5], rt[7], rt[8],
               ip, it, None]
        for s, src in enumerate(sel):
            if src is None:
                gh[s, :npos] = 1.0          # w slot
            else:
                gh[s, :npos] = src[pos].astype(np.float16)
        gh[18, npos:] = -30000.0            # iou_pred padding -> bce = 0

        cp = np.asarray(cls_pred[b], np.float32).reshape(10, N)
        f10h = np.ascontiguousarray(
            cp.astype(np.float16).reshape(10, P, FW).transpose(1, 0, 2))
        xt = cp[cf, np.arange(N)].astype(np.float16)
        xtch = np.stack([xt.reshape(P, FW),
                         cf.astype(np.float16).reshape(P, FW)], 1)
        maps.append({
            "g": np.ascontiguousarray(gh.reshape(NG, P, GW).transpose(1, 0, 2)),
            "f10": f10h,
            "xtc": np.ascontiguousarray(xtch),
        })
    return maps


def combine(parts):
    """parts: [8, 1, 8] per-core raw sums -> final [7] float32."""
    p = np.asarray(parts, np.float64).sum(0).reshape(8)
    focal_s, diou_s, z_s, h_s, v_s, bce_s, w_s = p[:7]
    num_pos = max(w_s, 1.0)
    cls_loss = focal_s / (8.0 * FW * P)
    bev_loss = diou_s / num_pos + 1.0
    z_loss = z_s / num_pos
    h_loss = h_s / num_pos
    vel_loss = v_s / num_pos
    iou_loss = bce_s / num_pos
    total = cls_loss + 2.0 * bev_loss + z_loss + h_loss + vel_loss + iou_loss
    return np.array([total, cls_loss, bev_loss, z_loss, h_loss, vel_loss,
                     iou_loss], np.float32)


def kernel(cls_pred, reg_pred, iou_pred, reg_targets, iou_targets,
           cls_targets, reg_weights, _trace=False):
    cls_pred, reg_pred, iou_pred, reg_targets, iou_targets, cls_targets, reg_weights = (
        np.asarray(a) for a in (cls_pred, reg_pred, iou_pred, reg_targets,
                                iou_targets, cls_targets, reg_weights))
    nc = _get_nc()
    in_maps = pack_inputs(cls_pred, reg_pred, iou_pred, reg_targets,
                          iou_targets, cls_targets, reg_weights)
    res = run_bass_kernel_spmd(nc, in_maps, core_ids=list(range(8)),
                               trace=_trace)
    parts = [res.results[i]["out"] for i in range(8)]
    out = combine(parts)
    if _trace:
        return out, res
    return out


# revision 25
# speedup vs baseline: 1.0368x; 1.0368x over previous
"""DetectionBEVLoss Trainium2 kernel: 8-core data-parallel (1 batch/core).

v2 design:
- Host compacts w>0 elements (geometry/sl1/bce run on [128, GW=272] instead
  of [128,512]); zero-padding contributes exactly 0 to every masked sum.
  Focal runs dense on all 65536 elements/core.
- Rotated IoU via midpoint Liang-Barsky: per box pair, 8 edge-pair-coords
  (slab, mbar, off, rho) built directly from center/trig products -- no
  corner tensors. Green's-theorem integral with constant-cross trick for
  the target-box direction.
- Custom fused DVE ops (8-deep ALU pipeline @ 1 elem/cycle/lane):
  2-NR reciprocal (stock RECIPROCAL_APPROX_FAST), seg=relu(min(H,1)+min(L,1)),
  fused smooth-L1+accumulate, clamped square-sum.
- ACT: sin/cos (table), exp, ln, abs, square, accumulations.
  Pool: class-sum avgpool for softmax denom + enclosing-box min/max chain.
- Host packs x_t = cls_pred[cls_t] (pure gather) so focal needs no
  10-way mask reduction on device.
"""
import math
import operator

import numpy as np

import concourse.bacc as bacc
import concourse.bass as bass
import concourse.mybir as mybir
import concourse.tile as tile
from concourse.bass_utils import run_bass_kernel_spmd

F16 = mybir.dt.float16
F32 = mybir.dt.float32
OP = mybir.AluOpType
AF = mybir.ActivationFunctionType

P = 128
FW = 512          # full free width (focal)
GW = 264          # compacted geometry width (33792 slots; ~32768 positives)
NG = 21           # geometry slots

# ---------------------------------------------------------------------------
# custom DVE ops: register into the concourse op table at import time.
# ---------------------------------------------------------------------------
from concourse import dve_ops as _dve_ops
from concourse.dve_ops import (
    DveOp,
    OPS as _OPS,
    RECIPROCAL_APPROX_FAST,
    RECIP_APPROX_FAST_CONSTS,
    _SUB_OPCODE_FOR_NAME,
    CUSTOM_DVE_SPECS,
)
from concourse.dve_spec import (
    Spec, Src0, Src1, C0, C2, One, Bin, AluOp, relu, sq, maxx, minn, lower,
    _has_src1,
)
from concourse.dve_uop import DveOpSpec


def _register(name, spec, subdim=False):
    if name in _SUB_OPCODE_FOR_NAME:
        return next(o for o in _OPS if o.name == name)
    row = max(_SUB_OPCODE_FOR_NAME.values()) + 1
    assert row < 0x20, "custom DVE opcode rows exhausted"
    uops = lower(spec, ver="v3")
    sp = DveOpSpec(name=name, opcode=row, uops=uops, rd1_en=_has_src1(spec))
    op = DveOp(name, spec, subdim=subdim, uops_sha={"v3": sp.sha("v3")})
    _OPS.append(op)
    _SUB_OPCODE_FOR_NAME[name] = row
    CUSTOM_DVE_SPECS[name] = spec
    return op


def _dve_minmax(a, b, is_min):
    # DVE MIN/MAX return the non-NaN operand
    a2 = np.where(np.isnan(a), b, a)
    b2 = np.where(np.isnan(b), a, b)
    return np.minimum(a2, b2) if is_min else np.maximum(a2, b2)


def _segrel_ref(in0, in1, s0, s1, imm2):
    m1 = _dve_minmax(in0.astype(np.float32), 1.0, True)
    m2 = _dve_minmax(in1.astype(np.float32), 1.0, True)
    return _dve_minmax(m1 + m2, 0.0, False)


def _sl1acc_ref(in0, in1, s0, s1, imm2):
    ad = np.abs(in0.astype(np.float32) - in1.astype(np.float32))
    m = np.minimum(ad, 1.0)
    b = ad * m + s0 * m * m
    return b, b.reshape(b.shape[0], -1).sum(axis=-1, keepdims=True)


def _sqsumm_ref(in0, in1, s0, s1, imm2):
    return np.maximum(in0.astype(np.float32) ** 2 + in1.astype(np.float32) ** 2,
                      imm2)


def _bce1_ref(in0, in1, s0, s1, imm2):
    x = in0.astype(np.float32)
    return np.maximum(x, 0) - x * in1.astype(np.float32)


_ad = Bin(AluOp.ABSOLUTE_DIFF, Src0, Src1)
_m = minn(_ad, One)
SEGREL = _register(
    "SEGREL_ANT",
    Spec(body=relu(minn(Src0, One) + minn(Src1, One)), reference=_segrel_ref))
SL1ACC = _register(
    "SL1ACC_ANT",
    Spec(body=_ad * _m + sq(_m) * C0, accum=operator.add,
         reference=_sl1acc_ref))
SQSUMM = _register(
    "SQSUMM_ANT",
    Spec(body=maxx(sq(Src0) + sq(Src1), C2), reference=_sqsumm_ref))
BCE1 = _register(
    "BCE1_ANT",
    Spec(body=relu(Src0) - Src0 * Src1, reference=_bce1_ref))
ABSMUL = _register(
    "ABSMUL_ANT",
    Spec(body=Bin(AluOp.ABSOLUTE_VALUE, Src0, Src0) * C0,
         reference=lambda in0, in1, s0, s1, imm2:
             np.abs(in0.astype(np.float32)) * s0))

RECIP_K = dict(s0=RECIP_APPROX_FAST_CONSTS["s0"],
               s1=RECIP_APPROX_FAST_CONSTS["s1"],
               imm2=RECIP_APPROX_FAST_CONSTS["imm2"])


def _ap(t, s0, slot_dims, col0, ncol, colstep=1):
    """Manual AP into tile t ([128, S, W]): base slot s0, then
    (slot_step, count) dims, innermost column dim."""
    ss = t.ap[-2][0]
    ap = [list(t.ap[0])] + [[s * ss, c] for s, c in slot_dims] + [[colstep, ncol]]
    return bass.AP(tensor=t.tensor, offset=t.offset + s0 * ss + col0, ap=ap)


def build_bass():
    nc = bacc.Bacc("TRN2", target_bir_lowering=False, debug=False)
    g = nc.declare_dram_parameter("g", [P, NG, GW], F16, isOutput=False)
    f10 = nc.declare_dram_parameter("f10", [P, 10, FW], F16, isOutput=False)
    xtc = nc.declare_dram_parameter("xtc", [P, 2, FW], F16, isOutput=False)
    outp = nc.declare_dram_parameter("out", [1, 8], F32, isOutput=True)

    with tile.TileContext(nc) as tc:
        with (
            tc.tile_pool(name="main", bufs=1) as pool,
            tc.tile_pool(name="small", bufs=1) as spool,
            tc.tile_pool(name="ps", bufs=1, space="PSUM") as ppool,
        ):
            G = pool.tile([P, NG, GW], F16)
            F10 = pool.tile([P, 10, FW], F16)
            XTC = pool.tile([P, 2, FW], F16)
            nc.sync.dma_start(out=G[:, 0:6, :], in_=g[:, 0:6, :])
            nc.sync.dma_start(out=G[:, 6:NG, :], in_=g[:, 6:NG, :])
            nc.sync.dma_start(out=F10, in_=f10[:, :, :])
            nc.sync.dma_start(out=XTC, in_=xtc[:, :, :])

            ones = spool.tile([P, 1], F32)
            nc.vector.memset(ones, 1.0)
            ACC = spool.tile([P, 8], F32)
            nc.vector.memset(ACC, 0.0)

            def const_col(val):
                t = spool.tile([P, 1], F32)
                nc.vector.memset(t, val)
                return t

            LNIN = pool.tile([P, 2, FW], F16)
            LNOUT = pool.tile([P, 2, FW], F16)
            nc.vector.memset(LNIN[:, 1, :], 1.0)
            HALFPI = const_col(math.pi / 2)
            ONE_C = const_col(1.0)

            # ---------------- DVE: d3 = (dx, dy, dth) ----------------
            D3 = pool.tile([P, 3, GW], F16)
            nc.vector.tensor_tensor(out=D3, in0=G[:, 0:3, :], in1=G[:, 3:6, :],
                                    op=OP.subtract)

            # G-only work: fills DVE while ACT does trig
            SCR = pool.tile([P, 2, GW], F16)
            nc.vector._custom_dve(SL1ACC, out=SCR[:, 0, :], in0=G[:, 10, :],
                                  in1=G[:, 14, :], s0=-0.5,
                                  accum_out=ACC[:, 2:3])
            nc.vector._custom_dve(SL1ACC, out=SCR[:, 0, :], in0=G[:, 11, :],
                                  in1=G[:, 15, :], s0=-0.5,
                                  accum_out=ACC[:, 3:4])
            nc.vector._custom_dve(SL1ACC, out=SCR, in0=G[:, 12:14, :],
                                  in1=G[:, 16:18, :], s0=-0.5,
                                  accum_out=ACC[:, 4:5])
            D2T = pool.tile([P, GW], F16)
            nc.vector._custom_dve(SQSUMM, out=D2T, in0=D3[:, 0, :],
                                  in1=D3[:, 1, :], imm2=0.0)
            XIO = G[:, 18, :]
            T1B = pool.tile([P, GW], F16)
            nc.vector._custom_dve(BCE1, out=T1B, in0=XIO, in1=G[:, 19, :])

            # ---------------- ACT: trig ----------------
            # SC6 = [sp, st, sd, cp, ct, cd]
            SC6 = pool.tile([P, 6, GW], F16)
            TH2 = _ap(G, 2, [(3, 2)], 0, GW)          # (thp, tht)
            nc.scalar.activation(SC6[:, 0:2, :], TH2, AF.Sin)
            nc.scalar.activation(SC6[:, 2, :], D3[:, 2, :], AF.Sin)
            nc.scalar.activation(SC6[:, 3:5, :], TH2, AF.Sin, bias=HALFPI)
            nc.scalar.activation(SC6[:, 5, :], D3[:, 2, :], AF.Sin,
                                 bias=HALFPI)
            # ABS4 = [|cp|, |sp|, |ct|, |st|]
            ABS4 = pool.tile([P, 4, GW], F16)
            nc.scalar.activation(ABS4, _ap(SC6, 3, [(1, 2), (-3, 2)], 0, GW),
                                 AF.Abs)

            # ---------------- ACT: focal exp ----------------
            ET = pool.tile([P, 10, FW], F16)
            nc.scalar.activation(ET, F10, AF.Exp)
            ONEB = spool.tile([P, 1], F32)
            nc.vector.tensor_scalar(out=ONEB, in0=ET[:, 0, 0:1], scalar1=0.0,
                                    scalar2=1.0, op0=OP.mult, op1=OP.add)
            BA = pool.tile([P, GW], F16)
            nc.scalar.activation(BA, XIO, AF.Abs)
            nc.scalar.activation(BA, BA, AF.Exp, scale=-1.0)
            nc.scalar.activation(BA, BA, AF.Ln, bias=ONE_C, scale=ONEB)
            BJNK = pool.tile([P, GW], F16)
            nc.vector.tensor_tensor(out=T1B, in0=T1B, in1=BA, op=OP.add)
            nc.scalar.activation(BJNK, T1B, AF.Copy, accum_out=ACC[:, 5:6])
            nc.scalar.activation(BJNK, G[:, 20, :], AF.Copy,
                                 accum_out=ACC[:, 6:7])

            # ---------------- DVE: frames ----------------
            # FP8 = [ct*dx, st*dx, ct*dy, st*dy, cp*dx, sp*dx, cp*dy, sp*dy]
            FP8 = pool.tile([P, 8, GW], F16)
            DDUP = _ap(D3, 0, [(1, 2), (0, 2)], 0, GW)      # [dx, dx, dy, dy]
            nc.vector.tensor_tensor(
                out=FP8[:, 0:4, :],
                in0=_ap(SC6, 4, [(0, 2), (-3, 2)], 0, GW),   # [ct, st, ct, st]
                in1=DDUP, op=OP.mult)
            nc.vector.tensor_tensor(
                out=FP8[:, 4:8, :],
                in0=_ap(SC6, 3, [(0, 2), (-3, 2)], 0, GW),   # [cp, sp, cp, sp]
                in1=DDUP, op=OP.mult)
            # CB4 = [cBx, cBy, eAx, eAy]
            CB4 = pool.tile([P, 4, GW], F16)
            nc.vector.tensor_tensor(out=_ap(CB4, 0, [(2, 2)], 0, GW),
                                    in0=_ap(FP8, 0, [(4, 2)], 0, GW),
                                    in1=_ap(FP8, 3, [(4, 2)], 0, GW), op=OP.add)
            nc.vector.tensor_tensor(out=_ap(CB4, 1, [(2, 2)], 0, GW),
                                    in0=_ap(FP8, 2, [(4, 2)], 0, GW),
                                    in1=_ap(FP8, 1, [(4, 2)], 0, GW),
                                    op=OP.subtract)

            # ---------------- DVE: p8 = edge half-vector components ----------
            # [a1cd, a1sd, -b1sd, b1cd, a2cd, -a2sd, b2sd, b2cd]
            DIM4 = pool.tile([P, 4, GW], F16)
            nc.vector.tensor_scalar(out=DIM4, in0=G[:, 6:10, :], scalar1=0.5,
                                    scalar2=None, op0=OP.mult)
            P8 = pool.tile([P, 8, GW], F16)
            CDb = _ap(SC6, 5, [(0, 2)], 0, GW)
            SDb = _ap(SC6, 2, [(0, 2)], 0, GW)
            DIMV = _ap(DIM4, 0, [(2, 2), (1, 2)], 0, GW)
            CDb2 = _ap(SC6, 5, [(0, 2), (0, 2)], 0, GW)
            SDb2 = _ap(SC6, 2, [(0, 2), (0, 2)], 0, GW)
            nc.vector.tensor_tensor(out=_ap(P8, 0, [(4, 2), (3, 2)], 0, GW),
                                    in0=DIMV, in1=CDb2, op=OP.mult)
            nc.vector.tensor_tensor(out=_ap(P8, 1, [(4, 2), (1, 2)], 0, GW),
                                    in0=DIMV, in1=SDb2, op=OP.mult)
            NEGV = _ap(P8, 2, [(3, 2)], 0, GW)
            nc.vector.tensor_scalar(out=NEGV, in0=NEGV, scalar1=-1.0,
                                    scalar2=None, op0=OP.mult)

            # ---------------- enclosing box / c2 / d2 (early) ----------------
            E8 = pool.tile([P, 8, GW], F16)
            nc.vector.tensor_tensor(
                out=E8, in0=_ap(DIM4, 0, [(2, 2), (0, 2), (1, 2)], 0, GW),
                in1=_ap(ABS4, 0, [(1, 4), (0, 2)], 0, GW), op=OP.mult)
            ES4 = pool.tile([P, 4, GW], F16)
            nc.vector.tensor_tensor(out=ES4, in0=_ap(E8, 0, [(2, 4)], 0, GW),
                                    in1=_ap(E8, 3, [(4, 2), (-2, 2)], 0, GW),
                                    op=OP.add)
            CEN = _ap(G, 0, [(3, 2), (1, 2)], 0, GW)    # [xp, yp, xt, yt]
            XE4 = pool.tile([P, 4, GW], F16)
            XD4 = pool.tile([P, 4, GW], F16)
            nc.vector.tensor_tensor(out=XE4, in0=CEN, in1=ES4, op=OP.add)
            nc.vector.tensor_tensor(out=XD4, in0=CEN, in1=ES4, op=OP.subtract)
            HX2 = pool.tile([P, 2, GW], F16)
            LX2 = pool.tile([P, 2, GW], F16)
            nc.vector.tensor_tensor(out=HX2, in0=XE4[:, 0:2, :],
                                    in1=XE4[:, 2:4, :], op=OP.max)
            nc.vector.tensor_tensor(out=LX2, in0=XD4[:, 0:2, :],
                                    in1=XD4[:, 2:4, :], op=OP.min)
            SP2 = pool.tile([P, 2, GW], F16)
            nc.vector.tensor_tensor(out=SP2, in0=HX2, in1=LX2, op=OP.subtract)
            C2C = pool.tile([P, GW], F16)
            nc.vector._custom_dve(SQSUMM, out=C2C, in0=SP2[:, 0, :],
                                  in1=SP2[:, 1, :], imm2=6e-5)
            RC2 = pool.tile([P, GW], F16)
            nc.vector._custom_dve(RECIPROCAL_APPROX_FAST, out=RC2, in0=C2C,
                                  **RECIP_K)
            DLT = pool.tile([P, GW], F16)
            nc.vector.tensor_tensor(out=DLT, in0=D2T, in1=RC2, op=OP.mult)

            JNK = pool.tile([P, GW], F16)

            # ---------------- DVE: reciprocals (2-NR) + clamp ----------------
            R8 = pool.tile([P, 8, GW], F16)
            nc.vector._custom_dve(RECIPROCAL_APPROX_FAST, out=R8, in0=P8,
                                  **RECIP_K)
            # min-first so NaN (from 1/0) lands at +8000
            nc.vector.tensor_scalar(out=R8, in0=R8, scalar1=8000.0,
                                    scalar2=-8000.0, op0=OP.min, op1=OP.max)

            # ---------------- ACT: |rho| (before ET so DVE isn't blocked) ----
            AR8 = pool.tile([P, 8, GW], F16)
            nc.scalar.activation(AR8, R8, AF.Abs)


            # ---------------- Pool: enclosing box + class-sum ----------------
            # (emitted later, after deps are defined)

            # ---------------- DVE: alpha/gamma/delta ----------------
            AL8 = pool.tile([P, 8, GW], F16)
            nc.vector.tensor_tensor(
                out=AL8, in0=_ap(DIM4, 2, [(-2, 2), (0, 2), (1, 2)], 0, GW),
                in1=AR8, op=OP.mult)
            GM8 = pool.tile([P, 8, GW], F16)
            nc.vector.tensor_tensor(
                out=GM8, in0=_ap(CB4, 0, [(2, 2), (0, 2), (1, 2)], 0, GW),
                in1=R8, op=OP.mult)
            DL8 = pool.tile([P, 8, GW], F16)
            nc.vector.tensor_tensor(
                out=DL8, in0=_ap(P8, 2, [(4, 2), (-2, 2), (1, 2)], 0, GW),
                in1=R8, op=OP.mult)

            A1T = pool.tile([P, 8, GW], F16)
            A2T = pool.tile([P, 8, GW], F16)
            nc.vector.tensor_tensor(out=A1T, in0=AL8, in1=GM8, op=OP.subtract)
            nc.vector.tensor_tensor(out=A2T, in0=AL8, in1=GM8, op=OP.add)
            HT = pool.tile([P, 16, GW], F16)
            LT = pool.tile([P, 16, GW], F16)
            nc.vector.tensor_tensor(out=HT[:, 0:8, :], in0=A1T, in1=DL8,
                                    op=OP.subtract)
            nc.vector.tensor_tensor(out=HT[:, 8:16, :], in0=A1T, in1=DL8,
                                    op=OP.add)
            nc.vector.tensor_tensor(out=LT[:, 0:8, :], in0=A2T, in1=DL8,
                                    op=OP.add)
            nc.vector.tensor_tensor(out=LT[:, 8:16, :], in0=A2T, in1=DL8,
                                    op=OP.subtract)

            SH8 = pool.tile([P, 8, GW], F16)
            SL8 = pool.tile([P, 8, GW], F16)
            nc.vector.tensor_tensor(out=SH8, in0=_ap(HT, 0, [(2, 8)], 0, GW),
                                    in1=_ap(HT, 1, [(2, 8)], 0, GW), op=OP.min)
            nc.vector.tensor_tensor(out=SL8, in0=_ap(LT, 0, [(2, 8)], 0, GW),
                                    in1=_ap(LT, 1, [(2, 8)], 0, GW), op=OP.min)
            SEG8 = pool.tile([P, 8, GW], F16)
            nc.vector._custom_dve(SEGREL, out=SEG8, in0=SH8, in1=SL8)

            # ---------------- DVE: focal chain (early, overlaps clip) ----
            T5 = pool.tile([P, 5, FW], F16)
            nc.vector.tensor_tensor(out=T5, in0=ET[:, 0:5, :],
                                    in1=ET[:, 5:10, :], op=OP.add)
            T2B = pool.tile([P, 2, FW], F16)
            nc.vector.tensor_tensor(out=T2B, in0=T5[:, 0:2, :],
                                    in1=T5[:, 2:4, :], op=OP.add)
            SAVG = pool.tile([P, FW], F16)
            nc.vector.tensor_tensor(out=SAVG, in0=T2B[:, 0, :],
                                    in1=T2B[:, 1, :], op=OP.add)
            nc.vector.tensor_tensor(out=SAVG, in0=SAVG, in1=T5[:, 4, :],
                                    op=OP.add)
            LNS = pool.tile([P, FW], F16)
            nc.scalar.activation(LNS, SAVG, AF.Ln)
            nc.vector.tensor_tensor(out=LNOUT[:, 0, :], in0=XTC[:, 0, :],
                                    in1=LNS, op=OP.subtract)
            PTT = pool.tile([P, FW], F16)
            nc.scalar.activation(PTT, LNOUT[:, 0, :], AF.Exp)
            OM2 = pool.tile([P, FW], F16)
            nc.scalar.activation(OM2, PTT, AF.Square, scale=-1.0,
                                 bias=ONE_C)
            ALPT = pool.tile([P, FW], F16)
            nc.vector.tensor_scalar(out=ALPT, in0=XTC[:, 1, :], scalar1=0.5,
                                    scalar2=-0.5, op0=OP.is_gt, op1=OP.mult)
            FF = pool.tile([P, FW], F16)
            nc.vector.tensor_tensor(out=FF, in0=OM2, in1=LNOUT[:, 0, :],
                                    op=OP.mult)
            nc.vector.scalar_tensor_tensor(out=FF, in0=ALPT, scalar=0.75,
                                           in1=FF, op0=OP.add, op1=OP.mult)
            JNKF = pool.tile([P, FW], F16)
            nc.scalar.activation(JNKF, FF, AF.Copy, scale=-1.0,
                                 accum_out=ACC[:, 0:1])


            # ---------------- DVE: integral ----------------
            PS4 = pool.tile([P, 4, GW], F16)
            nc.vector.tensor_tensor(out=PS4, in0=SEG8[:, 0:4, :],
                                    in1=SEG8[:, 4:8, :], op=OP.add)
            PD2 = pool.tile([P, 2, GW], F16)
            nc.vector.tensor_tensor(out=PD2, in0=SEG8[:, 0:2, :],
                                    in1=SEG8[:, 4:6, :], op=OP.subtract)
            SAB2 = pool.tile([P, 2, GW], F16)
            nc.vector.tensor_tensor(out=SAB2, in0=_ap(PS4, 0, [(2, 2)], 0, GW),
                                    in1=_ap(PS4, 1, [(2, 2)], 0, GW), op=OP.add)
            CP4 = pool.tile([P, 4, GW], F16)
            nc.vector.tensor_tensor(out=CP4,
                                    in0=_ap(CB4, 0, [(0, 2), (1, 2)], 0, GW),
                                    in1=_ap(P8, 1, [(2, 2), (-1, 2)], 0, GW),
                                    op=OP.mult)
            CX2 = pool.tile([P, 2, GW], F16)
            nc.vector.tensor_tensor(out=CX2, in0=_ap(CP4, 0, [(2, 2)], 0, GW),
                                    in1=_ap(CP4, 1, [(2, 2)], 0, GW),
                                    op=OP.subtract)
            M2 = pool.tile([P, 2, GW], F16)
            nc.vector.tensor_tensor(out=M2, in0=CX2, in1=PD2, op=OP.mult)
            AB2 = pool.tile([P, 2, GW], F16)
            nc.vector.tensor_tensor(out=AB2, in0=_ap(DIM4, 0, [(2, 2)], 0, GW),
                                    in1=_ap(DIM4, 1, [(2, 2)], 0, GW),
                                    op=OP.mult)
            IAB2 = pool.tile([P, 2, GW], F16)
            nc.vector.tensor_tensor(out=IAB2, in0=AB2, in1=SAB2, op=OP.mult)
            IA1 = pool.tile([P, GW], F16)
            nc.vector.tensor_tensor(out=IA1, in0=M2[:, 1, :], in1=M2[:, 0, :],
                                    op=OP.subtract)
            nc.vector.tensor_tensor(out=IA1, in0=IA1, in1=IAB2[:, 0, :],
                                    op=OP.add)
            nc.vector.tensor_tensor(out=IA1, in0=IA1, in1=IAB2[:, 1, :],
                                    op=OP.add)
            INTER = pool.tile([P, GW], F16)
            nc.vector._custom_dve(ABSMUL, out=INTER, in0=IA1, s0=0.5)

            # ---------------- DVE: union + iou ----------------
            USUM = pool.tile([P, GW], F16)
            nc.vector.tensor_tensor(out=USUM, in0=AB2[:, 0, :],
                                    in1=AB2[:, 1, :], op=OP.add)
            U = pool.tile([P, GW], F16)
            nc.vector.scalar_tensor_tensor(out=U, in0=USUM, scalar=4.0,
                                           in1=INTER, op0=OP.mult,
                                           op1=OP.subtract)
            nc.vector.tensor_scalar(out=U, in0=U, scalar1=6e-5, scalar2=None,
                                    op0=OP.max)
            RU = pool.tile([P, GW], F16)
            nc.vector._custom_dve(RECIPROCAL_APPROX_FAST, out=RU, in0=U,
                                  **RECIP_K)
            IOU = pool.tile([P, GW], F16)
            nc.vector.tensor_tensor(out=IOU, in0=INTER, in1=RU, op=OP.mult)
            nc.vector.tensor_tensor(out=DLT, in0=DLT, in1=IOU, op=OP.subtract)
            nc.scalar.activation(JNK, DLT, AF.Copy, accum_out=ACC[:, 1:2])





            # ---------------- cross-partition reduce + out ----------------
            PS = ppool.tile([1, 8], F32)
            nc.tensor.matmul(PS, ones, ACC, start=True, stop=True)
            OUT = spool.tile([1, 8], F32)
            nc.scalar.copy(out=OUT, in_=PS)
            nc.sync.dma_start(out=outp[:, :], in_=OUT)
    nc.compile()
    return nc


_NC_CACHE = None


def _get_nc():
    global _NC_CACHE
    if _NC_CACHE is None:
        _NC_CACHE = build_bass()
    return _NC_CACHE


def pack_inputs(cls_pred, reg_pred, iou_pred, reg_targets, iou_targets,
                cls_targets, reg_weights):
    """Returns list of 8 per-core input dicts."""
    B = cls_pred.shape[0]
    N = FW * P
    maps = []
    for b in range(B):
        rp = np.asarray(reg_pred[b], np.float32).reshape(9, N)
        rt = np.asarray(reg_targets[b], np.float32).reshape(9, N)
        ip = np.asarray(iou_pred[b], np.float32).reshape(N)
        it = np.asarray(iou_targets[b], np.float32).reshape(N)
        cf = np.asarray(cls_targets[b]).reshape(N)
        wf = np.asarray(reg_weights[b]).reshape(N)

        pos = np.flatnonzero(wf > 0)
        npos = pos.size
        assert npos <= P * GW, f"core {b}: {npos} positives > {P * GW}"

        gh = np.zeros((NG, P * GW), np.float16)
        sel = [rp[0], rp[1], rp[6], rt[0], rt[1], rt[6],
               rp[4], rp[3], rt[4], rt[3],
               rp[2], rp[5], rp[7], rp[8],
               rt[2], rt[5], rt[7], rt[8],
               ip, it, None]
        for s, src in enumerate(sel):
            if src is None:
                gh[s, :npos] = 1.0          # w slot
            else:
                gh[s, :npos] = src[pos].astype(np.float16)
        gh[18, npos:] = -30000.0            # iou_pred padding -> bce = 0

        cp = np.asarray(cls_pred[b], np.float32).reshape(10, N)
        f10h = np.ascontiguousarray(
            cp.astype(np.float16).reshape(10, P, FW).transpose(1, 0, 2))
        xt = cp[cf, np.arange(N)].astype(np.float16)
        xtch = np.stack([xt.reshape(P, FW),
                         cf.astype(np.float16).reshape(P, FW)], 1)
        maps.append({
            "g": np.ascontiguousarray(gh.reshape(NG, P, GW).transpose(1, 0, 2)),
            "f10": f10h,
            "xtc": np.ascontiguousarray(xtch),
        })
    return maps


def combine(parts):
    """parts: [8, 1, 8] per-core raw sums -> final [7] float32."""
    p = np.asarray(parts, np.float64).sum(0).reshape(8)
    focal_s, diou_s, z_s, h_s, v_s, bce_s, w_s = p[:7]
    num_pos = max(w_s, 1.0)
    cls_loss = focal_s / (8.0 * FW * P)
    bev_loss = diou_s / num_pos + 1.0
    z_loss = z_s / num_pos
    h_loss = h_s / num_pos
    vel_loss = v_s / num_pos
    iou_loss = bce_s / num_pos
    total = cls_loss + 2.0 * bev_loss + z_loss + h_loss + vel_loss + iou_loss
    return np.array([total, cls_loss, bev_loss, z_loss, h_loss, vel_loss,
                     iou_loss], np.float32)


def kernel(cls_pred, reg_pred, iou_pred, reg_targets, iou_targets,
           cls_targets, reg_weights, _trace=False):
    cls_pred, reg_pred, iou_pred, reg_targets, iou_targets, cls_targets, reg_weights = (
        np.asarray(a) for a in (cls_pred, reg_pred, iou_pred, reg_targets,
                                iou_targets, cls_targets, reg_weights))
    nc = _get_nc()
    in_maps = pack_inputs(cls_pred, reg_pred, iou_pred, reg_targets,
                          iou_targets, cls_targets, reg_weights)
    res = run_bass_kernel_spmd(nc, in_maps, core_ids=list(range(8)),
                               trace=_trace)
    parts = [res.results[i]["out"] for i in range(8)]
    out = combine(parts)
    if _trace:
        return out, res
    return out
